# revision 16
# baseline (speedup 1.0000x reference)
"""Multi-head self-attention with RoPE — Trainium2 Bass kernel, 8 NeuronCores.

Sharding: core c = 2*b + g handles batch b = c//2 and head-group g = c%2
(8 of the 16 heads).  No cross-core collectives: each core projects its
own half of the heads through the matching w_out row block into a
partial y [L, E] (fp32), and the host sums the two partials per batch.
Decoupling the cores keeps each NEFF's execution window free of
cross-core waits (collectives couple exec time to SPMD launch skew).

Per-core dataflow (matmuls bf16, fp32 PSUM accumulation):
  xT [E, L] bf16 (pre-transposed on host)
  QKV:   Q^T/K^T pair tiles via W-stationary matmuls; V natural [L, 512].
  RoPE:  weights pre-permuted on host to de-interleave even/odd dims, so
         rotate-half becomes a 32-partition block swap (SBUF->SBUF DMA);
         cos/sin multiplies + combine add on DVE.
  Attention (lq-outer, pair-inner):
    Scores: S^T half-tiles [Lk=128, Lq=512]; the two heads of a pair
         share a [128, 1024] PSUM tile (double buffered) per Lk chunk so
         exp runs as one wide ACT instruction overlapped with PE.
    Softmax: denominator via ones-column appended to V (PSUM partition 64
         of O^T); reciprocal_approx + gpsimd partition_broadcast.
    AV:  O^T[65, 512] += V_aug^T A^T over 16 Lk chunks; per-head O^T
         accumulators [65, 512] double buffered (psum: 4+2+2 = 8 banks).
    Normalized O^T lands in o2_sb [128, pair, L] (odd head moved to
         partitions 64-127 by a small SBUF->SBUF DMA) — proj-ready.
  Proj:  y[128-row chunk] = o2^T.T @ w_out_own [512, E], injected into
         the next lq tile's attention units so PE fills ACT-bound slack;
         partial y DMA'd out per chunk.
"""

import contextlib
import functools

import numpy as np
import ml_dtypes

import concourse.bass as bass
import concourse.mybir as mybir
import concourse.tile as tile
from concourse import bacc
from concourse.bass_utils import run_bass_kernel_spmd

BF16 = mybir.dt.bfloat16
F32 = mybir.dt.float32
N_CORES = 8
ROPE_THETA = 10000.0

B_FULL, L_FULL, E_FULL = 4, 2048, 1024
H_FULL = 16


def _emit3(tc, nc, xT, wqkv, wout, cosT, sinT, y, L, E, HC, D):
    P = 128
    EC = E // P                 # E chunks of 128 (contraction)
    NPAIR = HC // 2             # head pairs per core
    LT = L // 512               # 512-wide L tiles
    LKC = L // P                # 128-wide Lk chunks
    A = HC * D                  # local attention width (512)
    scale = 1.0 / float(np.sqrt(D))
    Exp = mybir.ActivationFunctionType.Exp

    ctx = contextlib.ExitStack()
    pool = ctx.enter_context(tc.tile_pool(name="sb", bufs=1))
    psum = ctx.enter_context(tc.tile_pool(name="ps", bufs=1, space="PSUM"))
    work = ctx.enter_context(tc.tile_pool(name="wk", bufs=1))

    # ---- persistent SBUF buffers ----
    xt_sb = pool.tile([P, EC, L], BF16, tag="xbuf")
    wqkv_sb = pool.tile([P, EC, 3 * A], BF16, tag="wqkv")
    wout_sb = pool.tile([P, A // P, E], BF16, tag="wout")   # own head rows
    cos_sb = pool.tile([P, L], BF16, tag="costab")
    sin_sb = pool.tile([P, L], BF16, tag="sintab")
    qk_sb = pool.tile([P, 2, NPAIR, L], BF16, tag="qk")      # [pair-rows, q/k, pair, L]
    vaug_sb = pool.tile([P, LKC, HC, D + 1], BF16, tag="vaug")
    o2_sb = pool.tile([P, NPAIR, L], BF16, tag="o2")         # normalized O^T, proj-ready

    # Input DMAs spread across both HWDGE rings (sync=SP, scalar=ACT) and
    # SWDGE (gpsimd) so the first V matmul only waits for the V-column
    # slice of w_qkv plus the first x chunk (~4µs), not the full 8.4MB.
    wq = wqkv.ap().rearrange("(c p) n -> p c n", p=P)
    nc.sync.dma_start(wqkv_sb[:, :, 2 * A : 3 * A], wq[:, :, 2 * A : 3 * A])
    for xc in range(LT):  # x in L-chunks so the V matmuls start early
        nc.scalar.dma_start(
            xt_sb[:, :, xc * 512 : (xc + 1) * 512],
            xT.ap()[:, xc * 512 : (xc + 1) * 512].rearrange("(c p) l -> p c l", p=P),
        )
    nc.sync.dma_start(wqkv_sb[:, :, 0 : 2 * A], wq[:, :, 0 : 2 * A])
    nc.gpsimd.dma_start(cos_sb[:], cosT.ap())
    nc.gpsimd.dma_start(sin_sb[:], sinT.ap())
    nc.gpsimd.dma_start(wout_sb[:], wout.ap().rearrange("(c p) n -> p c n", p=P))

    # ones column for the softmax denominator
    nc.vector.memset(vaug_sb[:, :, :, D : D + 1], 1.0)

    # ---- V = x @ Wv, natural [L, A] layout, 2 L-chunks per PSUM tile ----
    for vg in range(LKC // 2):
        ps = psum.tile([P, 1024], F32, tag="sc", bufs=2)
        for i in range(2):
            lt = vg * 2 + i
            for e in range(EC):
                nc.tensor.matmul(
                    ps[:, i * 512 : (i + 1) * 512],
                    lhsT=xt_sb[:, e, lt * P : (lt + 1) * P],
                    rhs=wqkv_sb[:, e, 2 * A : 3 * A],
                    start=(e == 0),
                    stop=(e == EC - 1),
                )
        nc.vector.tensor_copy(
            out=vaug_sb[:, vg * 2 : (vg + 1) * 2, :, 0:D],
            in_=ps[:].rearrange("p (t h d) -> p t h d", h=HC, d=D),
        )

    # ---- one Q or K 512-wide tile + RoPE (chain on DVE + swap DMAs) ----
    def qk_piece(p, lt, half):
        ps = psum.tile([P, 1024], F32, tag="sc", bufs=2)
        wcol = half * A + p * P
        for e in range(EC):
            nc.tensor.matmul(
                ps[:, 0:512],
                lhsT=wqkv_sb[:, e, wcol : wcol + P],
                rhs=xt_sb[:, e, lt * 512 : (lt + 1) * 512],
                start=(e == 0),
                stop=(e == EC - 1),
            )
        Lsl = slice(lt * 512, (lt + 1) * 512)
        qs = work.tile([P, 512], BF16, tag="qs", bufs=3)
        nc.vector.tensor_copy(out=qs[:], in_=ps[:, 0:512])
        w = work.tile([P, 512], BF16, tag="w", bufs=3)
        t = work.tile([P, 512], BF16, tag="w", bufs=3)
        nc.vector.tensor_mul(w[:], qs[:], sin_sb[:, Lsl])
        nc.vector.tensor_mul(t[:], qs[:], cos_sb[:, Lsl])
        wsw = work.tile([P, 512], BF16, tag="wsw", bufs=2)
        for blk in range(4):
            sb = blk ^ 1  # swap 32-row blocks pairwise
            nc.sync.dma_start(
                wsw[blk * 32 : (blk + 1) * 32, :], w[sb * 32 : (sb + 1) * 32, :]
            )
        nc.vector.tensor_add(qk_sb[:, half, p, Lsl], t[:], wsw[:])

    # ---- output projection piece: y rows [lcol, lcol+128) ----
    def proj_piece(lq, q4):
        lcol = lq * 512 + q4 * P
        ps = psum.tile([P, 1024], F32, tag="sc", bufs=2)
        for eh in range(E // 512):
            for c in range(A // P):
                nc.tensor.matmul(
                    ps[:, eh * 512 : (eh + 1) * 512],
                    lhsT=o2_sb[:, c, lcol : lcol + P],
                    rhs=wout_sb[:, c, eh * 512 : (eh + 1) * 512],
                    start=(c == 0),
                    stop=(c == A // P - 1),
                )
        yt = work.tile([P, E], BF16, tag="yt", bufs=2)
        nc.vector.tensor_copy(out=yt[:], in_=ps[:, :E])
        nc.sync.dma_start(y.ap()[lcol : lcol + P, :], yt[:])

    # ---- attention unit: pair p, 512-wide query tile lq ----
    def unit(p, lq, inject=None):
        inject = inject or {}
        Lq = slice(lq * 512, (lq + 1) * 512)
        otA = psum.tile([65, 512], F32, tag="otA", bufs=2)
        otB = psum.tile([65, 512], F32, tag="otB", bufs=2)
        ots = (otA, otB)
        pss = {}

        def scores(g):
            ps = psum.tile([P, 1024], F32, tag="sc", bufs=2)
            pss[g] = ps
            for hh in range(2):
                nc.tensor.matmul(
                    ps[:, hh * 512 : (hh + 1) * 512],
                    lhsT=qk_sb[hh * 64 : (hh + 1) * 64, 1, p, g * P : (g + 1) * P],
                    rhs=qk_sb[hh * 64 : (hh + 1) * 64, 0, p, Lq],
                    start=True,
                    stop=True,
                )

        # scores staggered one group ahead of AV so PE never queues
        # behind the exp it feeds.
        scores(0)
        for g in range(LKC):
            if g + 1 < LKC:
                scores(g + 1)
            ps = pss.pop(g)
            at = work.tile([P, 1024], BF16, tag="at", bufs=4)
            nc.scalar.activation(at[:], ps[:], Exp, scale=scale)
            for hh in range(2):
                nc.tensor.matmul(
                    ots[hh][:],
                    lhsT=vaug_sb[:, g, 2 * p + hh, :],
                    rhs=at[:, hh * 512 : (hh + 1) * 512],
                    start=(g == 0),
                    stop=(g == LKC - 1),
                )
            for fn in inject.get(g, ()):
                fn()
        # softmax denominator: PSUM row 64 -> SBUF row 64 -> (DMA) row 0 ->
        # reciprocal -> broadcast to 64 partitions -> scale O^T.  Even head
        # lands in o2_sb[0:64] directly; odd head goes via a staging tile
        # and a partition-moving DMA into o2_sb[64:128].
        for hh, otp in ((0, otA), (1, otB)):
            den = work.tile([65, 512], F32, tag="den", bufs=2)
            nc.vector.tensor_copy(out=den[64:65, :], in_=otp[64:65, :])
            den0 = work.tile([1, 512], F32, tag="den0", bufs=2)
            nc.sync.dma_start(den0[0:1, :], den[64:65, :])
            rec0 = work.tile([1, 512], F32, tag="rec0", bufs=2)
            nc.vector.reciprocal_approx_fast(rec0[0:1, :], den0[0:1, :])
            rbc = work.tile([64, 512], F32, tag="rbc", bufs=2)
            nc.gpsimd.partition_broadcast(rbc[:], rec0[0:1, :])
            if hh == 0:
                nc.vector.tensor_mul(o2_sb[0:64, p, Lq], otp[0:64, :], rbc[:])
            else:
                tmp = work.tile([64, 512], BF16, tag="otmp", bufs=2)
                nc.vector.tensor_mul(tmp[:], otp[0:64, :], rbc[:])
                nc.sync.dma_start(o2_sb[64:128, p, Lq], tmp[:])

    # ---- schedule: lq-outer; QK+RoPE pieces and proj pieces of the
    # previous tile injected into attention units to fill PE under the
    # ACT-bound exp stream.
    #
    # prelude: K(p0,*), Q(p0,lq0), Q(p1,lq0)
    # unit (p,   lq0): K(p+1,*) spread; Q(p+2, lq0)     [p <= NPAIR-2]
    # unit (last,lq0): Q(*, lq1)
    # unit (p, lq>=1): proj(lq-1, p); Q(p, lq+1)
    inj_lists = {(p, lq): [] for p in range(NPAIR) for lq in range(LT)}

    for lt in range(LT):
        qk_piece(0, lt, 1)
    qk_piece(0, 0, 0)
    qk_piece(1, 0, 0)

    for p in range(NPAIR - 1):
        for lt in range(LT):
            inj_lists[(p, 0)].append(functools.partial(qk_piece, p + 1, lt, 1))
        if p + 2 < NPAIR:
            inj_lists[(p, 0)].append(functools.partial(qk_piece, p + 2, 0, 0))
    if LT > 1:
        for p2 in range(NPAIR):
            inj_lists[(NPAIR - 1, 0)].append(functools.partial(qk_piece, p2, 1, 0))
    for lq in range(1, LT):
        for p in range(NPAIR):
            inj_lists[(p, lq)].append(functools.partial(proj_piece, lq - 1, p))
            if lq + 1 < LT:
                inj_lists[(p, lq)].append(functools.partial(qk_piece, p, lq + 1, 0))

    for lq in range(LT):
        for p in range(NPAIR):
            fns = inj_lists[(p, lq)]
            n = len(fns)
            inj = {}
            for i, fn in enumerate(fns):
                g = min(LKC - 1, max(1, (i + 1) * LKC // (n + 1)))
                inj.setdefault(g, []).append(fn)
            unit(p, lq, inj)
    for q4 in range(4):
        proj_piece(LT - 1, q4)

    ctx.close()


@functools.lru_cache(maxsize=2)
def build_module(L=L_FULL, E=E_FULL, HC=H_FULL // 2, D=64, asserts=False):
    nc = bacc.Bacc(
        "TRN2",
        target_bir_lowering=False,
        debug=False,
        enable_asserts=asserts,
        num_devices=N_CORES,
    )
    A = HC * D
    xT = nc.dram_tensor("xT", [E, L], BF16, kind="ExternalInput")
    wqkv = nc.dram_tensor("wqkv", [E, 3 * A], BF16, kind="ExternalInput")
    wout = nc.dram_tensor("wout", [A, E], BF16, kind="ExternalInput")
    cosT = nc.dram_tensor("cosT", [128, L], BF16, kind="ExternalInput")
    sinT = nc.dram_tensor("sinT", [128, L], BF16, kind="ExternalInput")
    y = nc.dram_tensor("y", [L, E], BF16, kind="ExternalOutput")
    with tile.TileContext(nc) as tc:
        _emit3(tc, nc, xT, wqkv, wout, cosT, sinT, y, L, E, HC, D)
    nc.compile()
    return nc


def _rope_tables(L, D):
    """cos/sin tables matching the de-interleaved weight layout.

    32-granular: rows [0,32) = freqs 0-31 "x1" slots, rows [32,64) their
    "x2" partners; rotate-half = 32-row block swap.  sin is pre-signed
    (+ on x1 slots, - on x2 slots).
    """
    half = D // 2
    inv_freq = 1.0 / (ROPE_THETA ** (np.arange(0, D, 2, dtype=np.float64) / D))
    freqs = np.arange(L, dtype=np.float64)[None, :] * inv_freq[:, None]  # [32, L]
    cos32 = np.cos(freqs)
    sin32 = np.sin(freqs)
    bf = ml_dtypes.bfloat16
    cos = np.tile(cos32, (128 // half, 1)).astype(bf)
    sin_block = np.concatenate([sin32, -sin32], axis=0)  # [64, L]
    sin = np.tile(sin_block, (2, 1)).astype(bf)
    return cos, sin


def _deint_cols(base, h, D):
    """Column indices of head h (offset base) in deinterleaved order."""
    cols = base + h * D + np.arange(D)
    return np.concatenate([cols[0::2], cols[1::2]])


def make_core_inputs(x, w_qkv, w_out, H=H_FULL, D=64):
    """Per-core input dicts from the full (unsharded) fp32 inputs."""
    Bv, L, E = x.shape
    HC = H // (N_CORES // Bv)
    A_full = H * D
    bf = ml_dtypes.bfloat16
    cos, sin = _rope_tables(L, D)
    in_maps = []
    for c in range(N_CORES):
        b, g = c // 2, c % 2
        # own-half w_out rows (tensor-parallel split over heads)
        wout_bf = np.ascontiguousarray(
            w_out[g * (HC * D) : (g + 1) * (HC * D), :]
        ).astype(bf)
        xT = np.ascontiguousarray(x[b].T).astype(bf)
        qcols = []
        kcols = []
        vcols = []
        for p in range(HC // 2):
            for hh in range(2):
                h = g * HC + 2 * p + hh
                qcols.append(_deint_cols(0, h, D))
                kcols.append(_deint_cols(A_full, h, D))
        for hl in range(HC):
            h = g * HC + hl
            vcols.append(2 * A_full + h * D + np.arange(D))
        cols = np.concatenate(qcols + kcols + vcols)
        wqkv_c = np.ascontiguousarray(w_qkv[:, cols]).astype(bf)
        in_maps.append(
            {
                "xT": xT,
                "wqkv": wqkv_c,
                "wout": wout_bf,
                "cosT": cos[:, :L].copy(),
                "sinT": sin[:, :L].copy(),
            }
        )
    return in_maps


def assemble_output(core_ys, Bv, L, E):
    """Full [B, L, E] from per-core partial y: sum each batch pair."""
    out = np.empty((Bv, L, E), dtype=np.float32)
    for b in range(Bv):
        out[b] = np.asarray(core_ys[2 * b]).astype(np.float32) + np.asarray(
            core_ys[2 * b + 1]
        ).astype(np.float32)
    return out


def kernel(x, w_qkv, w_out):
    x = np.asarray(x)
    w_qkv = np.asarray(w_qkv)
    w_out = np.asarray(w_out)
    Bv, L, E = x.shape
    nc = build_module(L=L, E=E)
    in_maps = make_core_inputs(x, w_qkv, w_out)
    res = run_bass_kernel_spmd(nc, in_maps, core_ids=list(range(N_CORES)))
    return assemble_output([res.results[c]["y"] for c in range(N_CORES)], Bv, L, E)
